# revision 1
# baseline (speedup 1.0000x reference)
# Trainium2 Bass kernel for nn_MultiHeadGridAttention1d (multi-head grid attention).
# 8 cores = (batch 0..4) x (head-half): per-core 4 heads attention + partial proj;
# host sums the two partials per batch.
import os, sys
import numpy as np
import ml_dtypes

if '/opt/trn_rl_repo' not in sys.path:
    sys.path.insert(0, '/opt/trn_rl_repo')

import concourse.bass as bass
import concourse.tile as tile
from concourse import bacc, mybir
from concourse import bass_utils

NH, KD, HD, C = 8, 32, 64, 512
W0 = 12; W4 = W0**4; G3 = W0**3; T = G3//4
SCALE = KD ** -0.5
PT = 432; NPT = W4 // PT
bf16 = mybir.dt.bfloat16; f32 = mybir.dt.float32

def mk(ap, dims, off=0):
    return bass.AP(tensor=ap.tensor, offset=ap.offset + off, ap=dims)

def build_program():
    nc = bacc.Bacc("TRN2", target_bir_lowering=False, debug=False, num_devices=8)
    def din(name, shape, dt=bf16):
        return nc.dram_tensor(name, shape, dt, kind="ExternalInput").ap()
    xb    = din("xb", [4, 128, W4])
    wconv = din("wconv", [4, 128, 576])
    bconv = din("bconv", [640], f32)
    def scr(name, n, dt=bf16):
        return nc.dram_tensor(name, [int(n)], dt, kind="Internal").ap()
    q1d = nc.dram_tensor("q1d", [128*W4], bf16, kind="ExternalOutput").ap()
    q2d = nc.dram_tensor("q2d", [128*W4], bf16, kind="ExternalOutput").ap()
    vd  = nc.dram_tensor("vd", [256*W4], bf16, kind="ExternalOutput").ap()
    ksd = nc.dram_tensor("ksd", [64*W4], bf16, kind="ExternalOutput").ap()

    EXPT = mybir.ActivationFunctionType.Exp
    COPYT = mybir.ActivationFunctionType.Copy
    IDENT = mybir.ActivationFunctionType.Identity
    AL = mybir.AluOpType
    import contextlib
    ctx = contextlib.ExitStack()
    with tile.TileContext(nc) as tc, ctx:
        const = ctx.enter_context(tc.tile_pool(name="const", bufs=1))
        sb  = ctx.enter_context(tc.tile_pool(name="sb", bufs=3))
        big = ctx.enter_context(tc.tile_pool(name="big", bufs=1))
        ps  = ctx.enter_context(tc.tile_pool(name="ps", bufs=2, space="PSUM"))
        ps2 = ctx.enter_context(tc.tile_pool(name="ps2", bufs=4, space="PSUM"))

        # ---------------- conv ----------------
        bcol = const.tile([128, 5], f32)
        for mch in range(5):
            nc.sync.dma_start(bcol[:, mch:mch+1], mk(bconv, [[1, 128], [1, 1]], mch*128))
        wc = const.tile([128, 4, 576], bf16)
        for kch in range(4):
            nc.sync.dma_start(wc[:, kch, :], wconv[kch])
        for pt in range(NPT):
            xt = sb.tile([128, 4, PT], bf16, tag="xt")
            for kch in range(4):
                nc.sync.dma_start(xt[:, kch, :], mk(xb, [[W4, 128], [1, PT]], kch*128*W4 + pt*PT))
            for mch in range(5):
                n = 128 if mch < 4 else 64
                cps = ps.tile([128, PT], f32, tag="cps")
                for kch in range(4):
                    nc.tensor.matmul(cps[0:n, :], wc[:, kch, mch*128:mch*128+n],
                                     xt[:, kch, :], start=(kch == 0), stop=(kch == 3))
                ot = sb.tile([128, PT], bf16, tag="cot")
                nc.scalar.activation(ot[0:n], cps[0:n], IDENT, bias=bcol[0:n, mch:mch+1])
                if mch < 2:
                    nc.sync.dma_start(mk(q1d if mch == 0 else q2d,
                                         [[W4, 128], [1, PT]], pt*PT), ot[:])
                elif mch < 4:
                    nc.sync.dma_start(mk(vd, [[W4, 128], [1, PT]],
                                         (mch-2)*128*W4 + pt*PT), ot[:])
                else:
                    nc.sync.dma_start(mk(ksd, [[W4, 64], [1, PT]], pt*PT), ot[0:64])

        ctx.close()
    nc.compile()
    return nc


def build_program2():
    nc = bacc.Bacc("TRN2", target_bir_lowering=False, debug=False, num_devices=8)
    def din(name, shape, dt=bf16):
        return nc.dram_tensor(name, shape, dt, kind="ExternalInput").ap()
    yd    = din("yd", [4*HD*W4])
    wproj = din("wproj", [2, 128, 512])
    bproj = din("bproj", [512], f32)
    out   = nc.dram_tensor("out", [512, W4], bf16, kind="ExternalOutput").ap()
    IDENT = mybir.ActivationFunctionType.Identity
    import contextlib
    ctx = contextlib.ExitStack()
    with tile.TileContext(nc) as tc, ctx:
        const = ctx.enter_context(tc.tile_pool(name="const", bufs=1))
        sb  = ctx.enter_context(tc.tile_pool(name="sb", bufs=3))
        ps  = ctx.enter_context(tc.tile_pool(name="ps", bufs=4, space="PSUM"))
        wp = const.tile([128, 2, 512], bf16)
        for kch in range(2):
            nc.sync.dma_start(wp[:, kch, :], wproj[kch])
        pcol = const.tile([128, 4], f32)
        for mch in range(4):
            nc.sync.dma_start(pcol[:, mch:mch+1], mk(bproj, [[1, 128], [1, 1]], mch*128))
        for pt in range(NPT):
            rhs = sb.tile([128, 2, PT], bf16, tag="prhs")
            for kch in range(2):
                nc.sync.dma_start(rhs[:, kch, :],
                                  mk(yd, [[W4, 128], [1, PT]], kch*128*W4 + pt*PT))
            for mch in range(4):
                pps = ps.tile([128, PT], f32, tag="pps")
                for kch in range(2):
                    nc.tensor.matmul(pps[:], wp[:, kch, mch*128:(mch+1)*128],
                                     rhs[:, kch, :], start=(kch == 0), stop=(kch == 1))
                po = sb.tile([128, PT], bf16, tag="po")
                nc.scalar.activation(po[:], pps[:], IDENT, bias=pcol[:, mch:mch+1])
                nc.sync.dma_start(mk(out, [[W4, 128], [1, PT]], mch*128*W4 + pt*PT), po[:])
        ctx.close()
    nc.compile()
    return nc


def host_prep(inputs, core):
    f = np.float32
    b = core // 2; hh = core % 2
    heads = list(range(hh*4, hh*4+4))
    def qch(h, s): return slice((h*2+s)*KD, (h*2+s)*KD+KD)
    def vch(h): return slice(h*HD, h*HD+HD)
    qk1_w, qk1_g, qk1_b = inputs['qk1_w'], inputs['qk1_g'], inputs['qk1_b']
    qk2_w, qk2_g, qk2_b = inputs['qk2_w'], inputs['qk2_g'], inputs['qk2_b']
    v_w, v_g, v_b = inputs['v_w'], inputs['v_g'], inputs['v_b']
    Wq1 = np.concatenate([qk1_w[qch(h,0)] * qk1_g[qch(h,0)][:,None] for h in heads])
    bq1 = np.concatenate([qk1_b[qch(h,0)] for h in heads])
    Wq2 = np.concatenate([qk2_w[qch(h,0)] * qk2_g[qch(h,0)][:,None] for h in heads])
    bq2 = np.concatenate([qk2_b[qch(h,0)] for h in heads])
    Wk1 = sum(qk1_w[qch(h,1)] * qk1_g[qch(h,1)][:,None] for h in range(NH))
    bk1 = sum(qk1_b[qch(h,1)] for h in range(NH))
    Wk2 = sum(qk2_w[qch(h,1)] * qk2_g[qch(h,1)][:,None] for h in range(NH))
    bk2 = sum(qk2_b[qch(h,1)] for h in range(NH))
    Wv = np.concatenate([v_w[vch(h)] * v_g[vch(h)][:,None] for h in heads])
    bv = np.concatenate([v_b[vch(h)] for h in heads])
    # conv weight tensor: (4 kch, 128 c, 576 out) = lhsT
    Wall = np.concatenate([Wq1, Wq2, Wv, Wk1, Wk2], axis=0).astype(f)  # (576, 512)
    wconv = Wall.T.reshape(4, 128, 576).astype(ml_dtypes.bfloat16)
    bconv = np.zeros(640, f)
    bconv[0:128] = bq1; bconv[128:256] = bq2; bconv[256:512] = bv
    bconv[512:544] = bk1; bconv[544:576] = bk2
    # pe weights (g folded), per chunk cols: (128, 6)
    wpe_full = np.concatenate([inputs['pe_w'][h*HD:(h+1)*HD] *
                               inputs['pe_g'][h*HD:(h+1)*HD][:,None] for h in heads])  # (256,3)
    wpet = np.zeros((128, 6), f)
    wpet[:, 0:3] = wpe_full[0:128]; wpet[:, 3:6] = wpe_full[128:256]
    # proj
    cols = np.concatenate([np.arange(h*HD, (h+1)*HD) for h in heads])
    Wp = (inputs['proj_w'][:, cols] * inputs['proj_g'][:, None]).astype(f)  # (512, 256)
    wproj = Wp.T.reshape(2, 128, 512).astype(ml_dtypes.bfloat16)
    if core % 2 == 0:
        bproj = (inputs['proj_b'] + inputs['proj_g'] *
                 (inputs['proj_w'] @ inputs['pe_b'])).astype(f)
    else:
        bproj = np.zeros(512, f)
    xbf = inputs['x'][b].reshape(4, 128, W4).astype(ml_dtypes.bfloat16)
    return {"xb": xbf, "wconv": wconv, "bconv": bconv, "wpe": wpet,
            "wpe_full": wpe_full, "wproj": wproj, "bproj": bproj}

_PROG = None
_PROG2 = None

_ATTN_JIT = None

def _attn_math(q1, q2, v, ks, xp):
    # q1,q2 (N,4,KD,12,12,12,12); v (N,4,HD,...); ks (N,2,KD,...); xp = jnp or np
    ks1 = ks[:, 0]; ks2 = ks[:, 1]
    def sm(z, ax):
        z = z - z.max(axis=ax, keepdims=True)
        e = xp.exp(z)
        return e / e.sum(axis=ax, keepdims=True)
    es = lambda s, *a: xp.einsum(s, *a, optimize=True) if xp is np else xp.einsum(s, *a)
    a1 = sm(es('chdijkl,cdIjkl->chIijkl', q1, ks1) * SCALE, 3)
    a2 = sm(es('chdijkl,cdiJkl->chJijkl', q2, ks2) * SCALE, 4)
    a3 = sm(es('chdijkl,cdijKl->chKijkl', q2, ks2) * SCALE, 5)
    a4 = sm(es('chdijkl,cdijkL->chLijkl', q2, ks2) * SCALE, 6)
    s1 = es('chdijkl,chIijkl->chdIjkl', v, a1)
    s2 = es('chdIjkl,chJIjkl->chdIJkl', s1, a2)
    m  = es('chKIJkl,chLIJKl->chLIJkl', a3, a4)
    return es('chdIJkl,chLIJkl->chdIJkL', s2, m)

def _attn_all(q1s, q2s, vs, kss, wpe):
    """Stacked bf16 over cores: q1s/q2s (8,128,W4), vs (8,256,W4), kss (8,64,W4),
    wpe (8,256,3) f32. Returns yd+pe as (8, 4*HD*W4) bf16. jax-CPU jit, numpy fallback."""
    global _ATTN_JIT
    G = (W0,)*4
    def full(q1r, q2r, vr, ksr, wper, xp):
        f32c = lambda t: t.astype(np.float32) if xp is np else t.astype('float32')
        q1 = f32c(q1r).reshape((8, 4, KD)+G); q2 = f32c(q2r).reshape((8, 4, KD)+G)
        v = f32c(vr); ks = f32c(ksr).reshape((8, 2, KD)+G)
        pe = v * wper[:, :, 1:2]
        if xp is np:
            pe[:, :, 1:] += v[:, :, :-1] * wper[:, :, 0:1]
            pe[:, :, :-1] += v[:, :, 1:] * wper[:, :, 2:3]
        else:
            pe = pe.at[:, :, 1:].add(v[:, :, :-1] * wper[:, :, 0:1])
            pe = pe.at[:, :, :-1].add(v[:, :, 1:] * wper[:, :, 2:3])
        y = _attn_math(q1, q2, v.reshape((8, 4, HD)+G), ks, xp)
        y = y.reshape(8, 4*HD*W4) + pe.reshape(8, 4*HD*W4)
        return y.astype(ml_dtypes.bfloat16) if xp is np else y.astype('bfloat16')
    try:
        import jax
        cpu = jax.devices("cpu")[0]
        if _ATTN_JIT is None:
            import jax.numpy as jnp
            _ATTN_JIT = jax.jit(lambda a, b, c, d, w: full(a, b, c, d, w, jnp))
        with jax.default_device(cpu):
            args = [jax.device_put(t, cpu) for t in (q1s, q2s, vs, kss, wpe)]
            return np.asarray(_ATTN_JIT(*args))
    except Exception:
        return full(q1s, q2s, vs, kss, wpe, np)

def kernel(**inputs):
    global _PROG, _PROG2
    inputs = {k: np.asarray(v) for k, v in inputs.items()}
    if _PROG is None:
        _PROG = build_program()
        _PROG2 = build_program2()
    preps = [host_prep(inputs, c) for c in range(8)]
    in1 = [{k: p[k] for k in ("xb", "wconv", "bconv")} for p in preps]
    r1 = bass_utils.run_bass_kernel_spmd(_PROG, in1, core_ids=list(range(8)))
    q1s = np.stack([r1.results[c]["q1d"].reshape(128, W4) for c in range(8)])
    q2s = np.stack([r1.results[c]["q2d"].reshape(128, W4) for c in range(8)])
    vs  = np.stack([r1.results[c]["vd"].reshape(256, W4) for c in range(8)])
    kss = np.stack([r1.results[c]["ksd"].reshape(64, W4) for c in range(8)])
    wpe = np.stack([preps[c]["wpe_full"] for c in range(8)]).astype(np.float32)
    yds = _attn_all(q1s, q2s, vs, kss, wpe)
    in2 = []
    for c in range(8):
        in2.append({"yd": yds[c],
                    "wproj": preps[c]["wproj"], "bproj": preps[c]["bproj"]})
    r2 = bass_utils.run_bass_kernel_spmd(_PROG2, in2, core_ids=list(range(8)))
    out = np.zeros((4, C, W4), np.float32)
    for c in range(8):
        out[c // 2] += r2.results[c]["out"].reshape(C, W4).astype(np.float32)
    return out



# revision 14
# speedup vs baseline: 6.8123x; 6.8123x over previous
# Trainium2 Bass kernel for nn_MultiHeadGridAttention1d — fully fused on-device.
# 8 cores = (batch 0..3) x (head-half). Per core: AllGather x halves (pair),
# conv1x1 -> grid attention (4 axes) -> +pe -> proj partial -> ReduceScatter
# over the pair -> bf16 output (256 out-channels per core).
import os, sys, math
import numpy as np
import ml_dtypes

if '/opt/trn_rl_repo' not in sys.path:
    sys.path.insert(0, '/opt/trn_rl_repo')

import concourse.bass as bass
import concourse.tile as tile
from concourse import bacc, mybir
from concourse import bass_utils

NH, KD, HD, C = 8, 32, 64, 512
SCALE = KD ** -0.5
bf16 = mybir.dt.bfloat16
f32 = mybir.dt.float32
AL = None  # set lazily
PAIRS = [[0, 1], [2, 3], [4, 5], [6, 7]]


def mk(ap, dims, off=0):
    return bass.AP(tensor=ap.tensor, offset=ap.offset + off, ap=dims)


def dma4(nc, dst, src, dims, off=0):
    """4-dim DRAM gather [part, a, b, c] -> dst tile [P, A, B, C], looping dim a
    (DMA hardware handles at most 3 dims per side)."""
    (ps, pn), (s1, n1), rest = dims[0], dims[1], dims[2:]
    for t in range(n1):
        nc.sync.dma_start(dst[:, t], mk(src, [[ps, pn]] + rest, off + t * s1))


def dma4w(nc, dst, dims, off, src_tile):
    """4-dim DRAM scatter from tile [P, A, B, C], looping dim a."""
    (ps, pn), (s1, n1), rest = dims[0], dims[1], dims[2:]
    for t in range(n1):
        nc.sync.dma_start(mk(dst, [[ps, pn]] + rest, off + t * s1), src_tile[:, t])


def build_program(W0=12):
    global AL
    AL = mybir.AluOpType
    W4 = W0 ** 4
    W3 = W0 ** 3
    W2 = W0 * W0
    KL = W2              # number of (k,l) pairs == number of (i,j) pairs
    NG = 2 if KL > 128 else 1
    GP = KL // NG
    AW = 4 * W0          # A tensor row width (4 heads x W0 targets)
    EXPT = mybir.ActivationFunctionType.Exp
    IDENT = mybir.ActivationFunctionType.Identity
    COPY = mybir.ActivationFunctionType.Copy

    nc = bacc.Bacc("TRN2", target_bir_lowering=False, debug=False, num_devices=8)

    def din(name, shape, dt=bf16):
        return nc.dram_tensor(name, shape, dt, kind="ExternalInput").ap()

    def dint(name, shape, dt=bf16):
        return nc.dram_tensor(name, shape, dt, kind="Internal").ap()

    xh    = din("xh", [2, 128, W4])
    wconv = din("wconv", [4, 128, 576])
    bconv = din("bconv", [576], f32)
    ptap  = din("ptap", [3, 256])
    wproj = din("wproj", [2, 128, 512])
    bproj = din("bproj", [512], f32)
    ident = din("ident", [128, 128])

    xb  = dint("xb", [2, 128, W4])
    xf  = dint("xf", [4, 128, W4])
    cq1 = dint("cq1", [W4, 128])
    cq2 = dint("cq2", [W4, 128])
    cv  = dint("cv", [W4 + 2, 256])
    cks = dint("cks", [W4, 64])
    A1  = dint("A1", [W4, AW])
    A2  = dint("A2", [W4, AW])
    A3  = dint("A3", [W4, AW])
    A4  = dint("A4", [W4, AW])
    S1  = dint("S1", [W4, 256], f32)
    S2  = dint("S2", [W4, 256], f32)
    Mt  = dint("Mt", [W4, AW], f32)
    YT  = dint("YT", [W4, 256])
    PP  = dint("PP", [512, W4], f32)
    RSO = dint("RSO", [256, W4], f32)
    OUT = nc.dram_tensor("OUT", [256, W4], bf16, kind="ExternalOutput").ap()

    # position chunks for conv/proj (M <= 128)
    chunks = []
    s = 0
    while s < W4:
        m = min(128, W4 - s)
        chunks.append((s, m))
        s += m

    import contextlib
    with tile.TileContext(nc) as tc:
        # ---------- Phase 0: AllGather x ----------
        nc.sync.dma_start(xb, xh)
        nc.gpsimd.collective_compute(
            "AllGather", AL.bypass, replica_groups=PAIRS,
            ins=[xb.opt()], outs=[xf.opt()])

        # ---------- Phase 1: conv1x1 (transposed output) ----------
        with tc.tile_pool(name="cconst", bufs=1) as cc, \
             tc.tile_pool(name="csb", bufs=3) as sb, \
             tc.tile_pool(name="cout", bufs=3) as ob, \
             tc.tile_pool(name="cps", bufs=2, space="PSUM") as ps:
            wc = cc.tile([128, 4, 576], bf16)
            for k in range(4):
                nc.sync.dma_start(wc[:, k, :], wconv[k])
            biasT = cc.tile([128, 576], f32)
            nc.sync.dma_start(biasT, mk(bconv, [[0, 128], [1, 576]]))
            zt = cc.tile([1, 256], bf16)
            nc.vector.memset(zt[:], 0)
            nc.sync.dma_start(mk(cv, [[256, 1], [1, 256]], 0), zt[:])
            nc.sync.dma_start(mk(cv, [[256, 1], [1, 256]], (W4 + 1) * 256), zt[:])

            for (s0, m) in chunks:
                xt = sb.tile([128, 4, 128], bf16, tag="xt")
                for k in range(4):
                    nc.sync.dma_start(xt[:, k, 0:m],
                                      mk(xf, [[W4, 128], [1, m]], k * 128 * W4 + s0))
                pA = ps.tile([128, 128], f32, tag="pA")
                pB = ps.tile([128, 128], f32, tag="pB")
                pC = ps.tile([128, 256], f32, tag="pC")
                pD = ps.tile([128, 64], f32, tag="pD")
                for k in range(4):
                    st, sp = (k == 0), (k == 3)
                    nc.tensor.matmul(pA[0:m, :], xt[:, k, 0:m], wc[:, k, 0:128], start=st, stop=sp)
                    nc.tensor.matmul(pB[0:m, :], xt[:, k, 0:m], wc[:, k, 128:256], start=st, stop=sp)
                    nc.tensor.matmul(pC[0:m, :], xt[:, k, 0:m], wc[:, k, 256:512], start=st, stop=sp)
                    nc.tensor.matmul(pD[0:m, :], xt[:, k, 0:m], wc[:, k, 512:576], start=st, stop=sp)
                o1 = ob.tile([128, 128], bf16, tag="o1")
                o2 = ob.tile([128, 128], bf16, tag="o2")
                o3 = ob.tile([128, 256], bf16, tag="o3")
                o4 = ob.tile([128, 64], bf16, tag="o4")
                nc.vector.scalar_tensor_tensor(o1[0:m], pA[0:m], 1.0, biasT[0:m, 0:128], AL.mult, AL.add)
                nc.vector.scalar_tensor_tensor(o2[0:m], pB[0:m], 1.0, biasT[0:m, 128:256], AL.mult, AL.add)
                nc.vector.scalar_tensor_tensor(o3[0:m], pC[0:m], 1.0, biasT[0:m, 256:512], AL.mult, AL.add)
                nc.vector.scalar_tensor_tensor(o4[0:m], pD[0:m], 1.0, biasT[0:m, 512:576], AL.mult, AL.add)
                nc.sync.dma_start(mk(cq1, [[128, m], [1, 128]], s0 * 128), o1[0:m])
                nc.sync.dma_start(mk(cq2, [[128, m], [1, 128]], s0 * 128), o2[0:m])
                nc.sync.dma_start(mk(cv, [[256, m], [1, 256]], (s0 + 1) * 256), o3[0:m])
                nc.sync.dma_start(mk(cks, [[64, m], [1, 64]], s0 * 64), o4[0:m])

        # ---------- Phase 2: logits + softmax (A1..A4) ----------
        # phase defs: (qsrc, kcol, pstr, ostr, fstr, xstr, Adst, wr_perm)
        # grid strides (in grid positions): i: W3, j: W2, k: W0, l: 1
        # Query pos = g*GP*pstr + part*pstr + o*ostr + fb*fstr
        # Key pos   = same with fb-slot replaced by X*xstr (A1: X replaces fb/i)
        # Each A phase: for query (o, fb): targets X, contraction d.
        #   A1: part=(k,l) pstr=1,  o=j ostr=W2, fb=i fstr=W3, X->i-slot xstr=W3, ks1
        #   A2: part=(k,l) pstr=1,  o=i ostr=W3, fb=j fstr=W2, X->j-slot xstr=W2, ks2
        #   A3: part=(i,j) pstr=W2, o=l ostr=1,  fb=k fstr=W0, X->k-slot xstr=W0, ks2
        #   A4: part=(i,j) pstr=W2, o=k ostr=W0, fb=l fstr=1,  X->l-slot xstr=1,  ks2
        defs = [
            (cq1, 0,  1,  W2, W3, W3, A1),
            (cq2, 32, 1,  W3, W2, W2, A2),
            (cq2, 32, W2, 1,  W0, W0, A3),
            (cq2, 32, W2, W0, 1,  1,  A4),
        ]
        for (qsrc, kcol, pstr, ostr, fstr, xstr, Adst) in defs:
            with tc.tile_pool(name="asb", bufs=2) as asb, \
                 tc.tile_pool(name="awk", bufs=2) as awk:
                for g in range(NG):
                    for h in range(4):
                        qoff = g * GP * pstr * 128 + h * 32
                        koff = g * GP * pstr * 64 + kcol
                        Qt = asb.tile([GP, W0, W0, KD], bf16, tag="Qt")
                        dma4(nc, Qt, qsrc, [[pstr * 128, GP], [ostr * 128, W0],
                                            [fstr * 128, W0], [1, KD]], qoff)
                        Kt = asb.tile([GP, W0, W0, KD], bf16, tag="Kt")
                        dma4(nc, Kt, cks, [[pstr * 64, GP], [ostr * 64, W0],
                                           [xstr * 64, W0], [1, KD]], koff)
                        LG = awk.tile([GP, W0, W0, W0], f32, tag="LG")
                        for o in range(W0):
                            # P[fb, X, d] = Q[fb, d] * K[X, d]
                            Pt = awk.tile([GP, W0, W0, KD], bf16, tag="Pt")
                            q_in = Qt[:, o].unsqueeze(2).broadcast_to((GP, W0, W0, KD))
                            k_in = Kt[:, o].unsqueeze(1).broadcast_to((GP, W0, W0, KD))
                            nc.vector.tensor_tensor(Pt[:], q_in, k_in, AL.mult)
                            nc.vector.tensor_reduce(LG[:, o], Pt[:], mybir.AxisListType.X, AL.add)
                        Et = awk.tile([GP, W0, W0, W0], bf16, tag="Et")
                        nc.scalar.activation(Et[:], LG[:], EXPT, scale=SCALE)
                        # softmax normalizes over fb (the original query axis),
                        # not over the target X: D[part, o, X] = sum_fb E
                        Dt = awk.tile([GP, W0, W0], f32, tag="Dt")
                        nc.vector.tensor_reduce(Dt[:], Et.transpose([0, 1, 3, 2]),
                                                mybir.AxisListType.X, AL.add)
                        Rt = awk.tile([GP, W0, W0], f32, tag="Rt")
                        nc.vector.reciprocal(Rt[:], Dt[:])
                        At = awk.tile([GP, W0, W0, W0], bf16, tag="At")
                        r_in = Rt.unsqueeze(2).broadcast_to((GP, W0, W0, W0))
                        nc.vector.tensor_tensor(At[:], Et[:], r_in, AL.mult)
                        # write A: query pos = g/part/o/fb; col h*W0 + X
                        dma4w(nc, Adst, [[pstr * AW, GP], [ostr * AW, W0],
                                         [fstr * AW, W0], [1, W0]],
                              g * GP * pstr * AW + h * W0, At)

        # ---------- Phase 3: s1 = sum_i v * a1 ----------
        # out s1[e; I,j,k,l]; partitions (k,l), loop j, free (I, e, i)
        with tc.tile_pool(name="s1sb", bufs=2) as s1sb, \
             tc.tile_pool(name="s1wk", bufs=2) as s1wk:
            for g in range(NG):
                for h in range(4):
                    Vt = s1sb.tile([GP, W0, W0, HD], bf16, tag="Vt")
                    dma4(nc, Vt, cv, [[256, GP], [W2 * 256, W0], [W3 * 256, W0], [1, HD]],
                         (g * GP + 1) * 256 + h * HD)
                    Atl = s1sb.tile([GP, W0, W0, W0], bf16, tag="Atl")
                    dma4(nc, Atl, A1, [[AW, GP], [W2 * AW, W0], [W3 * AW, W0], [1, W0]],
                         g * GP * AW + h * W0)
                    for j in range(W0):
                        Pj = s1wk.tile([GP, W0, HD, W0], bf16, tag="Pj")
                        v_in = Vt[:, j].transpose([0, 2, 1]).unsqueeze(1) \
                            .broadcast_to((GP, W0, HD, W0))
                        a_in = Atl[:, j].transpose([0, 2, 1]).unsqueeze(2) \
                            .broadcast_to((GP, W0, HD, W0))
                        nc.vector.tensor_tensor(Pj[:], v_in, a_in, AL.mult)
                        Sj = s1wk.tile([GP, W0, HD], f32, tag="Sj")
                        nc.vector.tensor_reduce(Sj[:], Pj[:], mybir.AxisListType.X, AL.add)
                        nc.sync.dma_start(
                            mk(S1, [[256, GP], [W3 * 256, W0], [1, HD]],
                               (j * W2 + g * GP) * 256 + h * HD),
                            Sj[:])

        # ---------- Phase 4: s2 = sum_j s1 * a2(at i=I) ----------
        # out s2[e; I,J,k,l]; partitions (k,l), loop I, free (J, e, j)
        with tc.tile_pool(name="s2sb", bufs=2) as s2sb, \
             tc.tile_pool(name="s2wk", bufs=2) as s2wk:
            for g in range(NG):
                for h in range(4):
                    S1t = s2sb.tile([GP, W0, W0, HD], f32, tag="S1t")
                    dma4(nc, S1t, S1, [[256, GP], [W3 * 256, W0], [W2 * 256, W0], [1, HD]],
                         g * GP * 256 + h * HD)
                    Atl2 = s2sb.tile([GP, W0, W0, W0], bf16, tag="Atl2")
                    dma4(nc, Atl2, A2, [[AW, GP], [W3 * AW, W0], [W2 * AW, W0], [1, W0]],
                         g * GP * AW + h * W0)
                    for I in range(W0):
                        PI = s2wk.tile([GP, W0, HD, W0], f32, tag="PI")
                        s_in = S1t[:, I].transpose([0, 2, 1]).unsqueeze(1) \
                            .broadcast_to((GP, W0, HD, W0))
                        a_in = Atl2[:, I].transpose([0, 2, 1]).unsqueeze(2) \
                            .broadcast_to((GP, W0, HD, W0))
                        nc.vector.tensor_tensor(PI[:], s_in, a_in, AL.mult)
                        SI = s2wk.tile([GP, W0, HD], f32, tag="SI")
                        nc.vector.tensor_reduce(SI[:], PI[:], mybir.AxisListType.X, AL.add)
                        nc.sync.dma_start(
                            mk(S2, [[256, GP], [W2 * 256, W0], [1, HD]],
                               (I * W3 + g * GP) * 256 + h * HD),
                            SI[:])

        # ---------- Phase 5: m = sum_K a3 * a4 ----------
        # out m[L; I,J,k,l]; partitions (I,J), loop l, free (k, L, K)
        with tc.tile_pool(name="msb", bufs=2) as msb, \
             tc.tile_pool(name="mwk", bufs=2) as mwk:
            for g in range(NG):
                for h in range(4):
                    A3t = msb.tile([GP, W0, W0, W0], bf16, tag="A3t")
                    dma4(nc, A3t, A3, [[W2 * AW, GP], [AW, W0], [W0 * AW, W0], [1, W0]],
                         g * GP * W2 * AW + h * W0)
                    A4t = msb.tile([GP, W0, W0, W0], bf16, tag="A4t")
                    dma4(nc, A4t, A4, [[W2 * AW, GP], [AW, W0], [W0 * AW, W0], [1, W0]],
                         g * GP * W2 * AW + h * W0)
                    for l in range(W0):
                        Pm = mwk.tile([GP, W0, W0, W0], bf16, tag="Pm")
                        a3_in = A3t[:, l].unsqueeze(2).broadcast_to((GP, W0, W0, W0))
                        a4_in = A4t[:, l].transpose([0, 2, 1]).unsqueeze(1) \
                            .broadcast_to((GP, W0, W0, W0))
                        nc.vector.tensor_tensor(Pm[:], a3_in, a4_in, AL.mult)
                        Sm = mwk.tile([GP, W0, W0], f32, tag="Sm")
                        nc.vector.tensor_reduce(Sm[:], Pm[:], mybir.AxisListType.X, AL.add)
                        nc.sync.dma_start(
                            mk(Mt, [[W2 * AW, GP], [W0 * AW, W0], [1, W0]],
                               (g * GP * W2 + l) * AW + h * W0),
                            Sm[:])

        # ---------- Phase 6: y = sum_l s2 * m ----------
        # out y[e; I,J,k,L]; partitions (I,J), loop k, free (L, e, l)
        with tc.tile_pool(name="ysb", bufs=2) as ysb, \
             tc.tile_pool(name="ywk", bufs=2) as ywk:
            for g in range(NG):
                for h in range(4):
                    S2t = ysb.tile([GP, W0, W0, HD], f32, tag="S2t")
                    dma4(nc, S2t, S2, [[W2 * 256, GP], [W0 * 256, W0], [256, W0], [1, HD]],
                         g * GP * W2 * 256 + h * HD)
                    Mtt = ysb.tile([GP, W0, W0, W0], f32, tag="Mtt")
                    dma4(nc, Mtt, Mt, [[W2 * AW, GP], [W0 * AW, W0], [AW, W0], [1, W0]],
                         g * GP * W2 * AW + h * W0)
                    for k in range(W0):
                        Py = ywk.tile([GP, W0, HD, W0], f32, tag="Py")
                        s_in = S2t[:, k].transpose([0, 2, 1]).unsqueeze(1) \
                            .broadcast_to((GP, W0, HD, W0))
                        m_in = Mtt[:, k].transpose([0, 2, 1]).unsqueeze(2) \
                            .broadcast_to((GP, W0, HD, W0))
                        nc.vector.tensor_tensor(Py[:], s_in, m_in, AL.mult)
                        Sy = ywk.tile([GP, W0, HD], f32, tag="Sy")
                        nc.vector.tensor_reduce(Sy[:], Py[:], mybir.AxisListType.X, AL.add)
                        Yb = ywk.tile([GP, W0, HD], bf16, tag="Yb")
                        nc.scalar.activation(Yb[:], Sy[:], IDENT)
                        nc.sync.dma_start(
                            mk(YT, [[W2 * 256, GP], [256, W0], [1, HD]],
                               (g * GP * W2 + k * W0) * 256 + h * HD),
                            Yb[:])

        # ---------- Phase 7: proj (+pe) ----------
        with tc.tile_pool(name="pconst", bufs=1) as pc, \
             tc.tile_pool(name="psb", bufs=3) as psb, \
             tc.tile_pool(name="pwk", bufs=2) as pwk, \
             tc.tile_pool(name="pps", bufs=1, space="PSUM") as pps, \
             tc.tile_pool(name="ppt", bufs=2, space="PSUM") as ppt:
            wp = pc.tile([128, 2, 512], bf16)
            for k in range(2):
                nc.sync.dma_start(wp[:, k, :], wproj[k])
            pcol = pc.tile([128, 4], f32)
            nc.sync.dma_start(pcol, mk(bproj, [[1, 128], [128, 4]]))
            ptapT = pc.tile([128, 3, 256], bf16)
            nc.sync.dma_start(ptapT, mk(ptap, [[0, 128], [256, 3], [1, 256]]))
            idt = pc.tile([128, 128], bf16)
            nc.sync.dma_start(idt, ident)

            for (s0, m) in chunks:
                yc = psb.tile([128, 256], bf16, tag="yc")
                nc.sync.dma_start(yc[0:m], mk(YT, [[256, m], [1, 256]], s0 * 256))
                v0 = psb.tile([128, 256], bf16, tag="v0")
                v1 = psb.tile([128, 256], bf16, tag="v1")
                v2 = psb.tile([128, 256], bf16, tag="v2")
                nc.sync.dma_start(v0[0:m], mk(cv, [[256, m], [1, 256]], s0 * 256))
                nc.sync.dma_start(v1[0:m], mk(cv, [[256, m], [1, 256]], (s0 + 1) * 256))
                nc.sync.dma_start(v2[0:m], mk(cv, [[256, m], [1, 256]], (s0 + 2) * 256))
                pe0 = pwk.tile([128, 256], bf16, tag="pe0")
                nc.vector.tensor_tensor(pe0[0:m], v0[0:m], ptapT[0:m, 0], AL.mult)
                pe1 = pwk.tile([128, 256], bf16, tag="pe1")
                nc.vector.tensor_tensor(pe1[0:m], v1[0:m], ptapT[0:m, 1], AL.mult)
                nc.vector.tensor_tensor(pe0[0:m], pe0[0:m], pe1[0:m], AL.add)
                nc.vector.tensor_tensor(pe1[0:m], v2[0:m], ptapT[0:m, 2], AL.mult)
                nc.vector.tensor_tensor(pe0[0:m], pe0[0:m], pe1[0:m], AL.add)
                yp = pwk.tile([128, 256], bf16, tag="yp")
                nc.vector.tensor_tensor(yp[0:m], yc[0:m], pe0[0:m], AL.add)
                # transpose 2 halves -> [ch, pos]
                rhs = psb.tile([128, 2, 128], bf16, tag="rhs")
                for cb in range(2):
                    tp = ppt.tile([128, 128], bf16, tag="tp")
                    nc.tensor.transpose(tp[0:128, 0:m], yp[0:m, cb * 128:(cb + 1) * 128],
                                        idt[0:m, 0:m])
                    nc.scalar.activation(rhs[:, cb, 0:m], tp[:, 0:m], COPY)
                for oc in range(4):
                    pj = pps.tile([128, 128], f32, tag=f"pj{oc}")
                    for cb in range(2):
                        nc.tensor.matmul(pj[:, 0:m], wp[:, cb, oc * 128:(oc + 1) * 128],
                                         rhs[:, cb, 0:m], start=(cb == 0), stop=(cb == 1))
                    po = pwk.tile([128, 128], f32, tag=f"po{oc}")
                    nc.scalar.activation(po[:, 0:m], pj[:, 0:m], IDENT, bias=pcol[:, oc:oc + 1])
                    nc.sync.dma_start(mk(PP, [[W4, 128], [1, m]], oc * 128 * W4 + s0),
                                      po[:, 0:m])

        # ---------- Phase 8: ReduceScatter + cast ----------
        with tc.tile_pool(name="rsb", bufs=3) as rsb:
            nc.gpsimd.collective_compute(
                "ReduceScatter", AL.add, replica_groups=PAIRS,
                ins=[PP.opt()], outs=[RSO.opt()])
            NCC = 4 if W4 % 4 == 0 and W4 > 8192 else 1
            CW = W4 // NCC
            for rb in range(2):
                for cc in range(NCC):
                    off = rb * 128 * W4 + cc * CW
                    ct = rsb.tile([128, CW], f32, tag="ct")
                    nc.sync.dma_start(ct[:], mk(RSO, [[W4, 128], [1, CW]], off))
                    cb = rsb.tile([128, CW], bf16, tag="cb")
                    nc.scalar.activation(cb[:], ct[:], COPY)
                    nc.sync.dma_start(mk(OUT, [[W4, 128], [1, CW]], off), cb[:])

    nc.compile()
    return nc


def host_prep(inputs, core, W0=12):
    f = np.float32
    W4 = W0 ** 4
    b = core // 2
    hh = core % 2
    heads = list(range(hh * 4, hh * 4 + 4))

    def qch(h, s):
        return slice((h * 2 + s) * KD, (h * 2 + s) * KD + KD)

    def vch(h):
        return slice(h * HD, h * HD + HD)

    qk1_w, qk1_g, qk1_b = inputs['qk1_w'], inputs['qk1_g'], inputs['qk1_b']
    qk2_w, qk2_g, qk2_b = inputs['qk2_w'], inputs['qk2_g'], inputs['qk2_b']
    v_w, v_g, v_b = inputs['v_w'], inputs['v_g'], inputs['v_b']
    Wq1 = np.concatenate([qk1_w[qch(h, 0)] * qk1_g[qch(h, 0)][:, None] for h in heads])
    bq1 = np.concatenate([qk1_b[qch(h, 0)] for h in heads])
    Wq2 = np.concatenate([qk2_w[qch(h, 0)] * qk2_g[qch(h, 0)][:, None] for h in heads])
    bq2 = np.concatenate([qk2_b[qch(h, 0)] for h in heads])
    Wk1 = sum(qk1_w[qch(h, 1)] * qk1_g[qch(h, 1)][:, None] for h in range(NH))
    bk1 = sum(qk1_b[qch(h, 1)] for h in range(NH))
    Wk2 = sum(qk2_w[qch(h, 1)] * qk2_g[qch(h, 1)][:, None] for h in range(NH))
    bk2 = sum(qk2_b[qch(h, 1)] for h in range(NH))
    Wv = np.concatenate([v_w[vch(h)] * v_g[vch(h)][:, None] for h in heads])
    bv = np.concatenate([v_b[vch(h)] for h in heads])
    Wall = np.concatenate([Wq1, Wq2, Wv, Wk1, Wk2], axis=0).astype(f)  # (576, 512)
    wconv = np.ascontiguousarray(Wall.T.reshape(4, 128, 576)).astype(ml_dtypes.bfloat16)
    bconv = np.concatenate([bq1, bq2, bv, bk1, bk2]).astype(f)

    cols = np.concatenate([np.arange(h * HD, (h + 1) * HD) for h in heads])
    ptap = np.ascontiguousarray(
        (inputs['pe_w'][cols] * inputs['pe_g'][cols][:, None]).T).astype(ml_dtypes.bfloat16)  # (3,256)
    Wp = (inputs['proj_w'][:, cols] * inputs['proj_g'][:, None]).astype(f)  # (512, 256)
    wproj = np.ascontiguousarray(Wp.T.reshape(2, 128, 512)).astype(ml_dtypes.bfloat16)
    if hh == 0:
        bproj = (inputs['proj_b'] + inputs['proj_g'] *
                 (inputs['proj_w'] @ inputs['pe_b'])).astype(f)
    else:
        bproj = np.zeros(512, f)
    xhv = inputs['x'][b].reshape(2, 2, 128, W4)[hh].astype(ml_dtypes.bfloat16)
    ident = np.eye(128, dtype=ml_dtypes.bfloat16)
    return {"xh": xhv, "wconv": wconv, "bconv": bconv, "ptap": ptap,
            "wproj": wproj, "bproj": bproj, "ident": ident}


_PROG = None


def kernel(**inputs):
    global _PROG
    inputs = {k: np.asarray(v) for k, v in inputs.items()}
    if _PROG is None:
        _PROG = build_program(12)
    W4 = 12 ** 4
    preps = [host_prep(inputs, c) for c in range(8)]
    r = bass_utils.run_bass_kernel_spmd(_PROG, preps, core_ids=list(range(8)))
    out = np.empty((4, C, W4), np.float32)
    for b in range(4):
        out[b, 0:256] = r.results[2 * b]["OUT"].astype(np.float32)
        out[b, 256:512] = r.results[2 * b + 1]["OUT"].astype(np.float32)
    return out
